# revision 16
# baseline (speedup 1.0000x reference)
"""Trainium2 Bass kernel for GAT-style GNN message passing (8 NeuronCores).

Math (matches reference):
    z = concat([m_sim @ Wm.T, d_sim @ Wd.T])           # [N, F]
    e = leaky_relu(sum(z[src] * z[dst], -1), 0.2)      # [E]
    alpha = softmax of e over incoming edges of dst
    h = elu(segment_sum(alpha[:,None] * z[src], dst))  # [N, F]

Distribution: nodes range-partitioned over 8 cores (12500 each); each core
owns edges whose dst falls in its range.  Edges are laid out in a column
template shared by all cores (per dst-group of 128 nodes x src-half x
src-parity, padded to the max class size over cores) so a single SPMD
program serves all 8 cores.

Launch 1: sharded projection z_c = x_c @ W.T.
Launch 2 (per core): one DMA pair-gather stream fetches z[src] (512B
descriptors: two adjacent node rows per token, dodging the sub-512B DMA
throughput penalty; int16 pair indices via two half tables).  z[dst] is
rebuilt on-chip: one-hot B (DVE is_equal vs an iota row), transposed on the
PE (matmul with identity), expanded against the dst-group node rows (bf16
hi/lo pair keeps fp32-grade accuracy for the attention logits).  The
segment softmax sums S = B^T @ [ex*z_src | ex] run on the PE into PSUM and
accumulate per group in SBUF -- no dma_scatter_add anywhere.  Softmax max-
subtraction is replaced by a clamp at 80 (shift-invariant per segment).
"""

import numpy as np
from contextlib import ExitStack

import concourse.bass as bass
import concourse.tile as tile
from concourse import bacc, mybir
from concourse import bass_utils

try:
    import ml_dtypes
    BF16 = ml_dtypes.bfloat16
except Exception:  # pragma: no cover
    BF16 = np.float32

# ---- problem constants ----
N = 100000
F = 64
C = 8
NPC = N // C          # 12500
R = 12544             # padded rows per core chunk
G = 98                # dst groups of 128 per core
SLOPE = 0.2
DM = 256
HALF = 50000
PAIRS = 25088         # pair rows per half table (25000 real + pad)
JPAIR = 25000         # junk pair row (zeros)
UCOLS = 8             # max cols per gather call / compute unit

_nc_cache = {}


def _wrap16(idx16):
    n = idx16.shape[0]
    w = np.ascontiguousarray(idx16.reshape(n // 16, 16).T)
    return np.tile(w, (8, 1))


# --------------------------------------------------------------------------
# host-side template + per-core fill
# --------------------------------------------------------------------------

def _template(src, dst):
    """Uniform column template over all cores: colmax[g, h, p] plus the
    static schedule (gather calls, compute units)."""
    core = dst // NPC
    dloc = dst - core * NPC
    g = dloc // 128
    h = src // HALF
    p = (src % HALF) & 1
    key = ((core * G + g) * 2 + h) * 2 + p
    cnt = np.bincount(key, minlength=C * G * 4).reshape(C, G, 2, 2)
    colmax = np.ceil(cnt.max(axis=0) / 128).astype(np.int64)  # [G, 2, 2]

    # column order: h-major, then g, then p
    calls = []   # (half, col0, ncols)
    units = []   # (call_idx, col_in_call, gcol0, ucols, g, h,
                 #  runs[(rel_c0, rn, p)], first_of_gh, last_of_gh)
    col = 0
    for hh in range(2):
        ccol0 = col          # current call start
        cur_half = hh
        for gg in range(G):
            span = []
            for pp in range(2):
                nc_ = int(colmax[gg, hh, pp])
                if nc_:
                    span.append((col - 0, nc_, pp))
                    col += nc_
            if not span:
                continue
            gh_c0 = span[0][0]
            gh_cols = sum(s[1] for s in span)
            # split into units at call boundaries (call = UCOLS window
            # within this half, aligned to ccol0)
            u0 = gh_c0
            gh_end = gh_c0 + gh_cols
            first = True
            while u0 < gh_end:
                cwin = (u0 - ccol0) // UCOLS
                cend = ccol0 + (cwin + 1) * UCOLS
                u1 = min(gh_end, cend, u0 + UCOLS)
                runs = []
                for s0, sn, pp in span:
                    lo = max(s0, u0); hi = min(s0 + sn, u1)
                    if lo < hi:
                        runs.append((lo - u0, hi - lo, pp))
                units.append([cwin, u0 - (ccol0 + cwin * UCOLS), u0,
                              u1 - u0, gg, hh, runs, first,
                              u1 == gh_end])
                first = False
                u0 = u1
        # finalize calls for this half
        ncols_h = col - ccol0
        ncalls = (ncols_h + UCOLS - 1) // UCOLS
        for k in range(ncalls):
            c0 = ccol0 + k * UCOLS
            calls.append((hh, c0, min(UCOLS, ccol0 + ncols_h - c0)))
        # remap unit call ids to global call indices
        base = len(calls) - ncalls
        for u in units:
            if u[5] == hh and isinstance(u[0], int) and u[2] >= ccol0:
                pass
        for u in units:
            if u[5] == hh:
                u[0] = base + (u[2] - ccol0) // UCOLS
    ncols = col
    return colmax, calls, units, ncols


def _fill_core(src_c, dloc_c, colmax, ncols):
    """Token arrays for one core under the shared template."""
    g = dloc_c // 128
    h = src_c // HALF
    p = (src_c % HALF) & 1
    pidx = ((src_c % HALF) >> 1).astype(np.int16)
    seg = (dloc_c - g * 128).astype(np.int16)

    order = np.lexsort((p, g, h))
    g_s = g[order]; h_s = h[order]; p_s = p[order]
    pi_s = pidx[order]; sg_s = seg[order]
    key = (h_s * G + g_s) * 2 + p_s
    caps = np.transpose(colmax, (1, 0, 2)).reshape(-1) * 128  # [h,g,p]
    cnt = np.bincount(key, minlength=2 * G * 2)
    assert (cnt <= caps).all()
    starts = np.concatenate([[0], np.cumsum(caps)[:-1]])
    # position of each sorted edge: starts[key] + rank within class
    first = np.r_[True, key[1:] != key[:-1]]
    grp_start = np.maximum.accumulate(
        np.where(first, np.arange(len(key)), 0)) if len(key) else None
    T = int(caps.sum())
    assert T == ncols * 128
    pi_full = np.full(T, JPAIR, np.int16)
    sg_full = np.full(T, -1, np.int16)
    if len(key):
        rank = np.arange(len(key)) - grp_start
        pos = starts[key] + rank
        pi_full[pos] = pi_s
        sg_full[pos] = sg_s
    segm = np.ascontiguousarray(sg_full.reshape(ncols, 128).T)
    return pi_full, segm


# --------------------------------------------------------------------------
# launch 1: projection
# --------------------------------------------------------------------------

def _build_proj_nc():
    nc = bacc.Bacc("TRN2", target_bir_lowering=False, debug=False,
                   num_devices=C)
    xT = nc.dram_tensor("xT", [DM, R], mybir.dt.float32,
                        kind="ExternalInput").ap()
    wT = nc.dram_tensor("wT", [DM, F], mybir.dt.float32,
                        kind="ExternalInput").ap()
    z_out = nc.dram_tensor("z", [R, F], mybir.dt.float32,
                           kind="ExternalOutput").ap()

    with tile.TileContext(nc) as tc:
        with ExitStack() as ctx:
            wp = ctx.enter_context(tc.tile_pool(name="w", bufs=1))
            xp = ctx.enter_context(tc.tile_pool(name="x", bufs=1))
            pp = ctx.enter_context(tc.tile_pool(name="ps", bufs=4,
                                                space="PSUM"))
            op = ctx.enter_context(tc.tile_pool(name="o", bufs=2))

            wt = wp.tile([128, 2, F], mybir.dt.float32)
            for j in range(2):
                nc.sync.dma_start(wt[:, j, :], wT[j * 128:(j + 1) * 128, :])
            xt = xp.tile([128, 2, R], mybir.dt.float32)
            for j in range(2):
                nc.sync.dma_start(xt[:, j, :], xT[j * 128:(j + 1) * 128, :])

            ntiles = R // 128
            SB = 8
            z_r = z_out.rearrange("(t p) f -> p t f", p=128)
            for r0 in range(0, ntiles, SB):
                sb = min(SB, ntiles - r0)
                ot = op.tile([128, sb, F], mybir.dt.float32, tag="ot")
                for t in range(sb):
                    r = r0 + t
                    ps = pp.tile([128, F], mybir.dt.float32, tag="ps")
                    for j in range(2):
                        nc.tensor.matmul(
                            out=ps[:],
                            lhsT=xt[:, j, r * 128:(r + 1) * 128],
                            rhs=wt[:, j, :],
                            start=(j == 0), stop=(j == 1))
                    nc.scalar.copy(ot[:, t, :], ps[:])
                nc.sync.dma_start(z_r[:, r0:r0 + sb, :], ot[:])
    nc.compile()
    return nc


# --------------------------------------------------------------------------
# launch 2: edge phase
# --------------------------------------------------------------------------

def _build_edge_nc(ncols, calls, units):
    T = ncols * 128
    nc = bacc.Bacc("TRN2", target_bir_lowering=False, debug=False,
                   num_devices=C, num_swdge_queues=1)
    zv = nc.dram_tensor("zv", [2 * PAIRS, 128], mybir.dt.float32,
                        kind="ExternalInput").ap()
    znhi_d = nc.dram_tensor("znhi", [R, F], mybir.dt.bfloat16,
                            kind="ExternalInput").ap()
    znlo_d = nc.dram_tensor("znlo", [R, F], mybir.dt.bfloat16,
                            kind="ExternalInput").ap()
    pidx_d = nc.dram_tensor("pidx", [128, T // 16], mybir.dt.int16,
                            kind="ExternalInput").ap()
    seg_d = nc.dram_tensor("seg", [128, ncols], mybir.dt.int16,
                           kind="ExternalInput").ap()
    iota_d = nc.dram_tensor("iota", [128, 128 * UCOLS], mybir.dt.int16,
                            kind="ExternalInput").ap()
    id_d = nc.dram_tensor("idm", [128, 128], mybir.dt.bfloat16,
                          kind="ExternalInput").ap()
    h_out = nc.dram_tensor("h", [R, F], mybir.dt.float32,
                           kind="ExternalOutput").ap()

    with tile.TileContext(nc) as tc:
        with ExitStack() as ctx:
            cp = ctx.enter_context(tc.tile_pool(name="const", bufs=1))
            acc = ctx.enter_context(tc.tile_pool(name="acc", bufs=1))
            znp = ctx.enter_context(tc.tile_pool(name="zn", bufs=2))
            gp = ctx.enter_context(tc.tile_pool(name="g", bufs=9))
            bp = ctx.enter_context(tc.tile_pool(name="b", bufs=8))
            sp = ctx.enter_context(tc.tile_pool(name="s", bufs=8))
            hp = ctx.enter_context(tc.tile_pool(name="h", bufs=3))
            pp_s = ctx.enter_context(tc.tile_pool(name="ps_s", bufs=2,
                                                  space="PSUM"))
            pp_z = ctx.enter_context(tc.tile_pool(name="ps_z", bufs=2,
                                                  space="PSUM"))
            pp_b = ctx.enter_context(tc.tile_pool(name="ps_b", bufs=2,
                                                  space="PSUM"))

            pidx_t = cp.tile([128, T // 16], mybir.dt.int16)
            nc.sync.dma_start(pidx_t[:], pidx_d[:, :])
            seg_t = cp.tile([128, ncols], mybir.dt.int16)
            nc.sync.dma_start(seg_t[:], seg_d[:, :])
            iota_t = cp.tile([128, 128, UCOLS], mybir.dt.int16)
            nc.sync.dma_start(
                iota_t[:], iota_d.rearrange("p (i j) -> p i j", j=UCOLS))
            znhi_a = cp.tile([128, G, F], mybir.dt.bfloat16)
            nc.sync.dma_start(
                znhi_a[:], znhi_d.rearrange("(g p) f -> p g f", p=128))
            znlo_a = cp.tile([128, G, F], mybir.dt.bfloat16)
            nc.sync.dma_start(
                znlo_a[:], znlo_d.rearrange("(g p) f -> p g f", p=128))
            id_t = cp.tile([128, 128], mybir.dt.bfloat16)
            nc.sync.dma_start(id_t[:], id_d[:, :])

            s_acc = acc.tile([128, G, F + 1], mybir.dt.float32)
            nc.vector.memset(s_acc[:], 0.0)

            gtiles = {}
            cur = {"zn": None, "s": None}
            ucnt = [0]
            for (ci, cic, gcol0, ucols, gg, hh, runs, first,
                 last) in units:
                if ci not in gtiles:
                    hh_c, c0_c, nc_c = calls[ci]
                    zsrc = gp.tile([128, UCOLS, 128], mybir.dt.float32,
                                   tag="zsrc")
                    ntok = nc_c * 128
                    nc.gpsimd.dma_gather(
                        zsrc[:, 0:nc_c, :],
                        zv[hh_c * PAIRS:(hh_c + 1) * PAIRS, :],
                        pidx_t[:, c0_c * 8:(c0_c + nc_c) * 8],
                        ntok, ntok, 128, queue_num=0)
                    gtiles = {ci: zsrc}   # keep only current call tile
                zsrc = gtiles[ci]
                if first:
                    cur["s"] = pp_s.tile([128, F + 1], mybir.dt.float32,
                                         tag="S", name="S")
                zn_hi = znhi_a[:, gg, :]
                zn_lo = znlo_a[:, gg, :]
                s_ps = cur["s"]

                B = bp.tile([128, 128, UCOLS], mybir.dt.bfloat16, tag="B")
                nc.vector.tensor_tensor(
                    out=B[:, :, 0:ucols],
                    in0=seg_t[:, None, gcol0:gcol0 + ucols].to_broadcast(
                        [128, 128, ucols]),
                    in1=iota_t[:, :, 0:ucols],
                    op=mybir.AluOpType.is_equal)

                btp = pp_b.tile([128, UCOLS, 128], mybir.dt.float32,
                                tag="bt")
                for j in range(ucols):
                    nc.tensor.matmul(out=btp[:, j, :], lhsT=B[:, :, j],
                                     rhs=id_t[:], start=True, stop=True)
                bt = bp.tile([128, UCOLS, 128], mybir.dt.bfloat16,
                             tag="btc")
                nc.scalar.copy(bt[:, 0:ucols, :], btp[:, 0:ucols, :])
                zd = pp_z.tile([128, UCOLS, F], mybir.dt.float32, tag="zd")
                for j in range(ucols):
                    nc.tensor.matmul(out=zd[:, j, :], lhsT=bt[:, j, :],
                                     rhs=zn_hi[:], start=True, stop=False)
                    nc.tensor.matmul(out=zd[:, j, :], lhsT=bt[:, j, :],
                                     rhs=zn_lo[:], start=False, stop=True)

                e_t = sp.tile([128, UCOLS], mybir.dt.float32, tag="e")
                prod = sp.tile([128, UCOLS, F], mybir.dt.float32,
                               tag="prod")
                ucnt[0] += 1
                peng = nc.vector
                for rc0, rn, pp_ in runs:
                    off = pp_ * F
                    peng.tensor_mul(
                        prod[:, rc0:rc0 + rn, :],
                        zsrc[:, cic + rc0:cic + rc0 + rn, off:off + F],
                        zd[:, rc0:rc0 + rn, :])
                    nc.vector.tensor_reduce(
                        e_t[:, rc0:rc0 + rn], prod[:, rc0:rc0 + rn, :],
                        axis=mybir.AxisListType.X, op=mybir.AluOpType.add)
                es = sp.tile([128, UCOLS], mybir.dt.float32, tag="es")
                nc.vector.tensor_scalar_mul(es[:, 0:ucols], e_t[:, 0:ucols],
                                            SLOPE)
                nc.vector.tensor_tensor(out=es[:, 0:ucols],
                                        in0=es[:, 0:ucols],
                                        in1=e_t[:, 0:ucols],
                                        op=mybir.AluOpType.max)
                nc.vector.tensor_scalar_min(es[:, 0:ucols], es[:, 0:ucols],
                                            80.0)
                ex = sp.tile([128, UCOLS], mybir.dt.float32, tag="ex")
                nc.scalar.activation(ex[:, 0:ucols], es[:, 0:ucols],
                                     mybir.ActivationFunctionType.Exp)

                pay = sp.tile([128, UCOLS, F + 1], mybir.dt.bfloat16,
                              tag="pay")
                for rc0, rn, pp_ in runs:
                    off = pp_ * F
                    peng.tensor_mul(
                        pay[:, rc0:rc0 + rn, 0:F],
                        zsrc[:, cic + rc0:cic + rc0 + rn, off:off + F],
                        ex[:, rc0:rc0 + rn, None].to_broadcast(
                            [128, rn, F]))
                nc.vector.tensor_copy(pay[:, 0:ucols, F:F + 1],
                                      ex[:, 0:ucols, None])

                for j in range(ucols):
                    nc.tensor.matmul(out=s_ps[:], lhsT=B[:, :, j],
                                     rhs=pay[:, j, :], start=(first and
                                     j == 0), stop=(last and
                                     j == ucols - 1))
                if last:
                    nc.vector.tensor_add(s_acc[:, gg, :], s_acc[:, gg, :],
                                         s_ps[:])

            # ---- normalization + elu + store ----
            NB = 14
            h_r = h_out.rearrange("(g p) f -> p g f", p=128)
            for g0 in range(0, G, NB):
                nb = min(NB, G - g0)
                rec = hp.tile([128, NB], mybir.dt.float32, tag="rec")
                nc.vector.tensor_scalar_max(rec[:, 0:nb],
                                            s_acc[:, g0:g0 + nb, F], 1e-30)
                nc.vector.reciprocal(rec[:, 0:nb], rec[:, 0:nb])
                h = hp.tile([128, NB, F], mybir.dt.float32, tag="ht")
                nc.vector.tensor_mul(
                    h[:, 0:nb, :], s_acc[:, g0:g0 + nb, 0:F],
                    rec[:, 0:nb, None].to_broadcast([128, nb, F]))
                hneg = hp.tile([128, NB, F], mybir.dt.float32, tag="hneg")
                nc.vector.tensor_scalar_min(hneg[:, 0:nb, :], h[:, 0:nb, :],
                                            0.0)
                nc.scalar.activation(hneg[:, 0:nb, :], hneg[:, 0:nb, :],
                                     mybir.ActivationFunctionType.Exp)
                nc.vector.tensor_scalar_max(h[:, 0:nb, :], h[:, 0:nb, :],
                                            0.0)
                nc.vector.tensor_add(h[:, 0:nb, :], h[:, 0:nb, :],
                                     hneg[:, 0:nb, :])
                nc.vector.tensor_scalar_add(h[:, 0:nb, :], h[:, 0:nb, :],
                                            -1.0)
                nc.sync.dma_start(h_r[:, g0:g0 + nb, :], h[:, 0:nb, :])
    nc.compile()
    return nc


# --------------------------------------------------------------------------
# entry point
# --------------------------------------------------------------------------

def kernel(m_sim, d_sim, Wm, Wd, src, dst, _profile=None):
    m_sim = np.asarray(m_sim, dtype=np.float32)
    d_sim = np.asarray(d_sim, dtype=np.float32)
    Wm = np.asarray(Wm, dtype=np.float32)
    Wd = np.asarray(Wd, dtype=np.float32)
    src = np.asarray(src).astype(np.int64)
    dst = np.asarray(dst).astype(np.int64)

    colmax, calls, units, ncols = _template(src, dst)

    # ---- launch 1: projection ----
    if "proj" not in _nc_cache:
        _nc_cache["proj"] = _build_proj_nc()
    proj_nc = _nc_cache["proj"]

    x = np.concatenate([m_sim, d_sim], axis=0)
    wmT = np.ascontiguousarray(Wm.T)
    wdT = np.ascontiguousarray(Wd.T)
    in1 = []
    for c in range(C):
        xT_c = np.zeros((DM, R), dtype=np.float32)
        xT_c[:, :NPC] = x[c * NPC:(c + 1) * NPC].T
        in1.append({"xT": xT_c, "wT": wmT if c < 4 else wdT})
    r1 = bass_utils.run_bass_kernel_spmd(proj_nc, in1,
                                         core_ids=list(range(C)),
                                         **(_profile or {}))
    zs = [r1.results[c]["z"][:NPC] for c in range(C)]

    zv = np.zeros((2 * PAIRS, 128), dtype=np.float32)
    for h in range(2):
        zh = np.concatenate(zs[4 * h:4 * h + 4], axis=0)
        zv[h * PAIRS:h * PAIRS + HALF // 2] = zh.reshape(HALF // 2, 128)

    # ---- launch 2 ----
    skey = ("edge", ncols, len(calls), len(units))
    if skey not in _nc_cache:
        _nc_cache[skey] = _build_edge_nc(ncols, calls, units)
    edge_nc = _nc_cache[skey]

    iota = np.tile(np.repeat(np.arange(128, dtype=np.int16), UCOLS)[None, :],
                   (128, 1))
    idm = np.eye(128).astype(BF16)
    core = dst // NPC
    dloc = dst - core * NPC
    in2 = []
    for c in range(C):
        m = core == c
        pidx, seg = _fill_core(src[m], dloc[m], colmax, ncols)
        zown = np.zeros((R, F), dtype=np.float32)
        zown[:NPC] = zs[c]
        znhi = zown.astype(BF16)
        znlo = (zown - znhi.astype(np.float32)).astype(BF16)
        in2.append({"zv": zv, "znhi": znhi, "znlo": znlo,
                    "pidx": _wrap16(pidx), "seg": seg, "iota": iota,
                    "idm": idm})
    r2 = bass_utils.run_bass_kernel_spmd(edge_nc, in2,
                                         core_ids=list(range(C)),
                                         **(_profile or {}))
    h_full = np.concatenate([r2.results[c]["h"][:NPC] for c in range(C)],
                            axis=0)
    kernel._last_results = (r1, r2)
    kernel._last_ncs = (proj_nc, edge_nc)
    return h_full
